# revision 7
# baseline (speedup 1.0000x reference)
"""Trainium2 Bass kernel for nn_MultiHeadAttention (B=2, L=2048, D=1024, H=16).

Sharding: 8 cores = 2 batches x 4 head-groups (4 heads each, tensor parallel).
Host compacts masked-out key positions (mask==0 keys are removed, not masked),
pads to a 128 multiple. Per core:
  QT = (Wq_g @ xq^T + bq)/8            [256, 2048]   (fp32r, 2 partition tiles)
  KT = Wk_g @ xk_c^T + bk              [256, LK]     (fp32r)
  V  = xv_c @ Wv_g^T + bv (ones-aug)   [LK, 4x(64+1)] (bf16, per kpos-tile)
  S^T[kpos,q] = KT_h^T.T @ QT_h  (K=64, row-packed head pairs)
  P = exp(S^T + padmask)               (bf16, denominators via ones row of V)
  O^T_h = V_h.T @ P ; normalize by row 64 (recip broadcast via K=1 matmul)
  out_partial = O^T.T @ Wo_g^T         [2048, 1024]  (fp32 psum -> DRAM)
Host sums the 4 head-group partials per batch and adds bo.
"""
import sys

sys.path.insert(0, "/opt/trn_rl_repo")

import numpy as np

B, L, D = 2, 2048, 1024
NH, DK = 16, 64
N_CORES = 8
GROUPS = 4          # head groups (cores per batch)
DQ = D // GROUPS    # 256 dims per group
HL = 4              # heads per group
T = L               # query tokens per core

_CACHE = {}


def _build(LK):
    import concourse.bacc as bacc
    import concourse.mybir as mybir
    import concourse.tile as tile

    FR = mybir.dt.float32r
    F32 = mybir.dt.float32
    BF = mybir.dt.bfloat16
    AF = mybir.ActivationFunctionType

    NKT = LK // 128          # kpos tiles
    KB = [(i, min(1024, LK - i)) for i in range(0, LK, 1024)]  # key chunks
    QB = [(i, 1024) for i in range(0, T, 1024)]                # query chunks

    nc = bacc.Bacc("TRN2", target_bir_lowering=False, debug=False,
                   num_devices=N_CORES)

    xqT = nc.dram_tensor("xqT", [D, T], FR, kind="ExternalInput").ap()
    xkT = nc.dram_tensor("xkT", [D, LK], FR, kind="ExternalInput").ap()
    xvT = nc.dram_tensor("xvT", [D, LK], FR, kind="ExternalInput").ap()
    wq = nc.dram_tensor("wq", [D, DQ], FR, kind="ExternalInput").ap()
    wk = nc.dram_tensor("wk", [D, DQ], FR, kind="ExternalInput").ap()
    wv = nc.dram_tensor("wv", [D, DQ], FR, kind="ExternalInput").ap()
    wo = nc.dram_tensor("wo", [DQ, D], BF, kind="ExternalInput").ap()
    bqs = nc.dram_tensor("bqs", [128, 2], F32, kind="ExternalInput").ap()
    bks = nc.dram_tensor("bks", [128, 2], F32, kind="ExternalInput").ap()
    bvr = nc.dram_tensor("bvr", [1, DQ], FR, kind="ExternalInput").ap()
    padm = nc.dram_tensor("padm", [128, NKT], F32, kind="ExternalInput").ap()
    onesc = nc.dram_tensor("onesc", [128, HL], BF, kind="ExternalInput").ap()
    onesr = nc.dram_tensor("onesr", [1, 128], FR, kind="ExternalInput").ap()
    out = nc.dram_tensor("out", [T, D], F32, kind="ExternalOutput").ap()

    with tile.TileContext(nc) as tc:
        with tc.tile_pool(name="wsb", bufs=1) as wsb, \
             tc.tile_pool(name="per", bufs=1) as per, \
             tc.tile_pool(name="xs", bufs=2) as xsp, \
             tc.tile_pool(name="es", bufs=1) as esp, \
             tc.tile_pool(name="sm", bufs=2) as smp, \
             tc.tile_pool(name="pa", bufs=3, space="PSUM") as pap, \
             tc.tile_pool(name="pb", bufs=2, space="PSUM") as pbp:

            # ---- persistent loads
            twq = wsb.tile([128, 8, DQ], FR, tag="twq")
            twk = wsb.tile([128, 8, DQ], FR, tag="twk")
            twv = wsb.tile([128, 8, DQ], FR, tag="twv")
            two = wsb.tile([128, 2, D], BF, tag="two")
            tbq = wsb.tile([128, 2], F32, tag="tbq")
            tbk = wsb.tile([128, 2], F32, tag="tbk")
            tbvr = wsb.tile([1, DQ], FR, tag="tbvr")
            tpad = wsb.tile([128, NKT], F32, tag="tpad")
            tonesc = wsb.tile([128, HL], BF, tag="tonesc")
            tonesr = wsb.tile([1, 128], FR, tag="tonesr")
            nc.sync.dma_start(twq[:], wq.rearrange("(a p) m -> p a m", p=128))
            nc.sync.dma_start(twk[:], wk.rearrange("(a p) m -> p a m", p=128))
            nc.sync.dma_start(twv[:], wv.rearrange("(a p) m -> p a m", p=128))
            nc.sync.dma_start(two[:], wo.rearrange("(a p) n -> p a n", p=128))
            nc.sync.dma_start(tbq[:], bqs[:])
            nc.sync.dma_start(tbk[:], bks[:])
            nc.sync.dma_start(tbvr[:], bvr[:])
            nc.sync.dma_start(tpad[:], padm[:])
            nc.sync.dma_start(tonesc[:], onesc[:])
            nc.sync.dma_start(tonesr[:], onesr[:])

            # ---- persistent intermediates
            QT = [per.tile([128, T], FR, tag=f"QT{p}", name=f"QT{p}") for p in range(2)]
            KT = [per.tile([128, LK], FR, tag=f"KT{p}", name=f"KT{p}") for p in range(2)]
            Vt = per.tile([128, NKT, HL * 65], BF, tag="Vt")
            OT = [per.tile([128, T], BF, tag=f"OT{p}", name=f"OT{p}") for p in range(2)]

            xkT_r = xkT.rearrange("(a p) n -> p a n", p=128)
            xvT_r = xvT.rearrange("(a p) n -> p a n", p=128)
            xqT_r = xqT.rearrange("(a p) n -> p a n", p=128)

            # ---- K projection: KT[p] = twk[p].T @ xkT (+bk via DVE)
            for cb, cw in KB:
                xk_t = xsp.tile([128, 8, cw], FR, tag="xs")
                for kt in range(8):
                    nc.sync.dma_start(xk_t[:, kt, :], xkT_r[:, kt, cb:cb + cw])
                for p in range(2):
                    ps = pap.tile([128, 1024], F32, tag="pa")
                    for h0 in range(0, cw, 512):
                        hw = min(512, cw - h0)
                        for kt in range(8):
                            nc.tensor.matmul(
                                ps[:, h0:h0 + hw],
                                twk[:, kt, p * 128:(p + 1) * 128],
                                xk_t[:, kt, h0:h0 + hw],
                                start=(kt == 0), stop=(kt == 7))
                    nc.vector.tensor_scalar(
                        KT[p][:, cb:cb + cw], ps[:, 0:cw],
                        1.0, tbk[:, p:p + 1],
                        mybir.AluOpType.mult, mybir.AluOpType.add)

                # ---- V projection for this chunk's kpos tiles
                xv_t = xsp.tile([128, 8, cw], FR, tag="xs")
                for kt in range(8):
                    nc.sync.dma_start(xv_t[:, kt, :], xvT_r[:, kt, cb:cb + cw])
                for tl in range(cw // 128):
                    tt = cb // 128 + tl
                    psv = pap.tile([128, DQ], F32, tag="pa")
                    for kt in range(8):
                        nc.tensor.matmul(
                            psv[:], xv_t[:, kt, tl * 128:(tl + 1) * 128],
                            twv[:, kt, :], start=(kt == 0), stop=False)
                    # bias row via K=1 matmul with ones lhsT
                    nc.tensor.matmul(psv[:], tonesr[:, 0:128], tbvr[:],
                                     start=False, stop=True)
                    nc.vector.tensor_copy(
                        Vt[:, tt, :].rearrange("p (h c) -> p h c", h=HL)[:, :, 0:64],
                        psv[:].rearrange("p (h c) -> p h c", h=HL))
                # ones columns of V tiles
                for tl in range(cw // 128):
                    tt = cb // 128 + tl
                    nc.sync.dma_start(
                        Vt[:, tt, :].rearrange("p (h c) -> p h c", h=HL)[:, :, 64:65],
                        onesc.rearrange("p (h o) -> p h o", h=HL))

            # ---- main loop over query blocks
            for qb, qw in QB:
                # Q projection for this block
                xq_t = xsp.tile([128, 8, qw], FR, tag="xs")
                for kt in range(8):
                    nc.sync.dma_start(xq_t[:, kt, :], xqT_r[:, kt, qb:qb + qw])
                for p in range(2):
                    ps = pap.tile([128, 1024], F32, tag="pa")
                    for h0 in range(0, qw, 512):
                        for kt in range(8):
                            nc.tensor.matmul(
                                ps[:, h0:h0 + 512],
                                twq[:, kt, p * 128:(p + 1) * 128],
                                xq_t[:, kt, h0:h0 + 512],
                                start=(kt == 0), stop=(kt == 7))
                    nc.vector.tensor_scalar(
                        QT[p][:, qb:qb + qw], ps[:],
                        0.125, tbq[:, p:p + 1],
                        mybir.AluOpType.mult, mybir.AluOpType.add)

                for p in range(2):
                    # stage A: scores + exp, head pair row-packed
                    es = [[None] * NKT, [None] * NKT]
                    for kt in range(NKT):
                        pss = [pap.tile([128, 1024], F32, tag="pa", name=f"pss{i}")
                               for i in range(2)]
                        for h0 in range(0, qw, 512):
                            for hh in range(2):
                                r = hh * 64
                                nc.tensor.matmul(
                                    pss[hh][:, h0:h0 + 512],
                                    KT[p][r:r + 64, kt * 128:(kt + 1) * 128],
                                    QT[p][r:r + 64, qb + h0:qb + h0 + 512],
                                    start=True, stop=True,
                                    tile_position=(r, 0))
                        for hh in range(2):
                            e = esp.tile([128, 1024], BF, tag=f"es{hh}_{kt}")
                            es[hh][kt] = e
                            nc.scalar.activation(e[:], pss[hh][:], AF.Exp,
                                                 bias=tpad[:, kt:kt + 1])
                    # stage B per head, 512-wide halves
                    for hh in range(2):
                        hl = 2 * p + hh
                        r = hh * 64
                        for h0 in range(0, qw, 512):
                            pso = pbp.tile([65, 512], F32, tag="pb")
                            for kt in range(NKT):
                                nc.tensor.matmul(
                                    pso[:], Vt[:, kt, hl * 65:hl * 65 + 65],
                                    es[hh][kt][:, h0:h0 + 512],
                                    start=(kt == 0), stop=(kt == NKT - 1))
                            dn = smp.tile([1, 512], FR, tag="dn")
                            nc.scalar.activation(dn[:], pso[64:65, :], AF.Copy)
                            psr = pbp.tile([64, 512], F32, tag="pb")
                            nc.tensor.matmul(psr[:], tonesr[:, 0:64], dn[:],
                                             start=True, stop=True)
                            rc = smp.tile([64, 512], F32, tag="rc")
                            nc.vector.reciprocal(rc[:], psr[:])
                            nc.vector.tensor_mul(
                                OT[p][r:r + 64, qb + h0:qb + h0 + 512],
                                pso[0:64, :], rc[:])

                # output projection for this query block
                for tl in range(qw // 128):
                    tt = (qb + tl * 128)
                    pso = pap.tile([128, 1024], F32, tag="pa")
                    for nh in range(2):
                        for dvt in range(2):
                            nc.tensor.matmul(
                                pso[:, nh * 512:(nh + 1) * 512],
                                OT[dvt][:, tt:tt + 128],
                                two[:, dvt, nh * 512:(nh + 1) * 512],
                                start=(dvt == 0), stop=(dvt == 1))
                    ost = smp.tile([128, 1024], F32, tag="ost")
                    nc.vector.tensor_copy(ost[:], pso[:])
                    nc.sync.dma_start(out[tt:tt + 128, :], ost[:])

    nc.compile()
    return nc


def _exec(nc, in_maps):
    from concourse import bass2jax
    return bass2jax.run_bass_via_pjrt(nc, in_maps, n_cores=N_CORES)


def _prep(query, key, value, mask, Wq, bq, Wk, bk, Wv, bv, Wo, bo):
    """Host-side sharding. Returns (LK, in_maps, meta)."""
    f32 = np.float32
    q3 = np.asarray(query, f32).reshape(B, L, D)
    k3 = np.asarray(key, f32).reshape(B, L, D)
    v3 = np.asarray(value, f32).reshape(B, L, D)
    mask = np.asarray(mask)

    idxs = [np.nonzero(mask[b])[0] for b in range(B)]
    lens = [len(ix) for ix in idxs]
    LK = max(128, ((max(lens) + 127) // 128) * 128)

    xqT, xkT, xvT, padm = [], [], [], []
    for b in range(B):
        xqT.append(np.ascontiguousarray(q3[b].T))
        kk = np.zeros((LK, D), f32)
        vv = np.zeros((LK, D), f32)
        kk[:lens[b]] = k3[b][idxs[b]]
        vv[:lens[b]] = v3[b][idxs[b]]
        xkT.append(np.ascontiguousarray(kk.T))
        xvT.append(np.ascontiguousarray(vv.T))
        pm = np.zeros(LK, f32)
        pm[lens[b]:] = -30000.0
        padm.append(np.ascontiguousarray(pm.reshape(LK // 128, 128).T))

    Wq, bq = np.asarray(Wq, f32), np.asarray(bq, f32)
    Wk, bk = np.asarray(Wk, f32), np.asarray(bk, f32)
    Wv, bv = np.asarray(Wv, f32), np.asarray(bv, f32)
    Wo = np.asarray(Wo, f32)

    gm = {}
    for g in range(GROUPS):
        sl = slice(g * DQ, (g + 1) * DQ)
        gm[g] = dict(
            wq=np.ascontiguousarray(Wq[sl, :].T),
            wk=np.ascontiguousarray(Wk[sl, :].T),
            wv=np.ascontiguousarray(Wv[sl, :].T),
            wo=np.ascontiguousarray(Wo[:, sl].T),
            bqs=np.ascontiguousarray((bq[sl] / 8.0).reshape(2, 128).T),
            bks=np.ascontiguousarray(bk[sl].reshape(2, 128).T),
            bvr=np.ascontiguousarray(bv[sl].reshape(1, DQ)),
        )

    import ml_dtypes
    bf16 = np.dtype(ml_dtypes.bfloat16)
    ones_c = np.ones((128, HL), bf16)
    ones_r = np.ones((1, 128), f32)

    in_maps = []
    for c in range(N_CORES):
        b, g = c // GROUPS, c % GROUPS
        m = gm[g]
        in_maps.append({
            "xqT": xqT[b], "xkT": xkT[b], "xvT": xvT[b],
            "wq": m["wq"], "wk": m["wk"], "wv": m["wv"],
            "wo": m["wo"].astype(bf16),
            "bqs": m["bqs"], "bks": m["bks"], "bvr": m["bvr"],
            "padm": padm[b], "onesc": ones_c, "onesr": ones_r,
        })
    return LK, in_maps


def kernel(query, key, value, mask, Wq, bq, Wk, bk, Wv, bv, Wo, bo):
    LK, in_maps = _prep(query, key, value, mask, Wq, bq, Wk, bk, Wv, bv, Wo, bo)
    if LK not in _CACHE:
        _CACHE[LK] = _build(LK)
    nc = _CACHE[LK]
    results = _exec(nc, in_maps)
    bo = np.asarray(bo, np.float32)
    out = np.zeros((B, L, D), np.float32)
    for c in range(N_CORES):
        out[c // GROUPS] += results[c]["out"]
    out += bo[None, None, :]
    return out
